# revision 4
# baseline (speedup 1.0000x reference)
"""MiniTransformer block on 8 Trainium2 NeuronCores.

Sharding: pure data-parallel over batch (B=8 -> 1 batch element per core,
no collectives). Per core the full transformer block (LN -> single-head
attention -> residual -> LN -> MLP -> residual) runs as one Bass/Tile kernel.

Key design points:
  * All matmuls run in float32r (TF32-like, 1 cycle/row on the PE at free
    dim >= 256 vs 4 cycles/row for fp32; measured fro rel err ~1.5e-4).
  * Activations for matmul consumption are kept transposed ([feature, token])
    so projections chain without transposes; only LN outputs are transposed
    (PE transpose, 4 per 128-row chunk).
  * Softmax: scores are computed transposed [t, s]; exp (with the 1/sqrt(D)
    scale fused) happens on the ScalarE during PSUM eviction; no max
    subtraction (LN-bounded scores, fp32 exp range is ample); the
    denominator comes from an extra ones-column appended to v, landing in
    PSUM as a per-partition scalar; normalization + residual add fold into
    a single scalar_tensor_tensor eviction.
  * Weight preprocessing (LN gamma/beta folding, Wv@Wo fusion) is constant
    folding done host-side in float64.
"""

import numpy as np

S, D, F, P = 2048, 512, 2048, 128
SC, DC, FC = S // P, D // P, F // P  # 16, 4, 16
SB = 256                             # s-block for attention and MLP
NB = S // SB                         # 8
CPB = SB // P                        # s-chunks per block = 2
NCORES = 8
LN_EPS = 1e-5
ATTN_SCALE = float(1.0 / np.sqrt(np.float32(D)))

_CACHE = {}


def _build():
    import concourse.mybir as mybir
    import concourse.tile as tile
    from concourse import bacc
    from concourse.masks import make_identity
    from contextlib import ExitStack

    f32 = mybir.dt.float32
    f32r = mybir.dt.float32r
    AF = mybir.ActivationFunctionType
    OP = mybir.AluOpType

    nc = bacc.Bacc("TRN2", target_bir_lowering=False, debug=False,
                   num_devices=NCORES)

    x_d = nc.dram_tensor("x", [S, D], f32, kind="ExternalInput").ap()
    wq_d = nc.dram_tensor("wq", [D, D], f32r, kind="ExternalInput").ap()
    wk_d = nc.dram_tensor("wk", [D, D], f32r, kind="ExternalInput").ap()
    wvo_d = nc.dram_tensor("wvo", [D, D], f32r, kind="ExternalInput").ap()
    w1_d = nc.dram_tensor("w1", [D, F], f32r, kind="ExternalInput").ap()
    w2_d = nc.dram_tensor("w2", [F, D], f32r, kind="ExternalInput").ap()
    bq_d = nc.dram_tensor("bq", [D], f32, kind="ExternalInput").ap()
    bk_d = nc.dram_tensor("bk", [D], f32, kind="ExternalInput").ap()
    bvo_d = nc.dram_tensor("bvo", [D], f32, kind="ExternalInput").ap()
    bf_d = nc.dram_tensor("bf", [F], f32, kind="ExternalInput").ap()
    b2_d = nc.dram_tensor("b2", [D], f32, kind="ExternalInput").ap()
    out_d = nc.dram_tensor("out", [S, D], f32, kind="ExternalOutput").ap()

    x_r = x_d.rearrange("(sc p) d -> p sc d", p=P)      # [128, 16, 512]
    out_r = out_d.rearrange("(sc p) d -> p sc d", p=P)
    wq_r = wq_d.rearrange("(ko ki) n -> ki ko n", ki=P)  # [128, 4, 512]
    wk_r = wk_d.rearrange("(ko ki) n -> ki ko n", ki=P)
    wvo_r = wvo_d.rearrange("(ko ki) n -> ki ko n", ki=P)
    w1_r = w1_d.rearrange("(ko ki) n -> ki ko n", ki=P)  # [128, 4, 2048]
    w2_r = w2_d.rearrange("(ko ki) n -> ki ko n", ki=P)  # [128, 16, 512]
    import concourse.bass as bass
    bq_r = bq_d.rearrange("(o p) -> p o", p=P)           # [128, 4]
    bk_r = bk_d.rearrange("(o p) -> p o", p=P)
    bf_r = bf_d.rearrange("(o p) -> p o", p=P)           # [128, 16]
    bvo_b = bass.AP(tensor=bvo_d.tensor, offset=bvo_d.offset,
                    ap=[[0, P], [1, D]])                 # partition-broadcast
    b2_b = bass.AP(tensor=b2_d.tensor, offset=b2_d.offset,
                   ap=[[0, P], [1, D]])

    with tile.TileContext(nc) as tc, ExitStack() as top:
        long_pool = top.enter_context(tc.tile_pool(name="long", bufs=1))
        const_pool = top.enter_context(tc.tile_pool(name="consts", bufs=1))

        # ---- constants / small tiles -------------------------------------
        ident_f = const_pool.tile([P, P], f32)
        make_identity(nc, ident_f[:])
        ident = const_pool.tile([P, P], f32r)
        nc.vector.tensor_copy(ident[:], ident_f[:])
        eps_t = const_pool.tile([P, 1], f32)
        nc.vector.memset(eps_t[:], LN_EPS)
        ones2 = const_pool.tile([P, 2], f32)
        nc.vector.memset(ones2[:, 0:1], 1.0)
        nc.vector.memset(ones2[:, 1:2], 0.0)
        bq_t = const_pool.tile([P, DC], f32)
        nc.sync.dma_start(bq_t[:], bq_r)
        bk_t = const_pool.tile([P, DC], f32)
        nc.sync.dma_start(bk_t[:], bk_r)
        bf_t = const_pool.tile([P, FC], f32)
        nc.sync.dma_start(bf_t[:], bf_r)
        bvo_t = const_pool.tile([P, D], f32)
        nc.sync.dma_start(bvo_t[:], bvo_b)
        b2_t = const_pool.tile([P, D], f32)
        nc.sync.dma_start(b2_t[:], b2_b)

        # ---- persistent activations --------------------------------------
        xb = long_pool.tile([P, SC, D], f32)             # x, becomes x2 in place
        hT = long_pool.tile([P, DC, S], f32r, tag="actT")  # hT, reused as h2T

        def layer_norm_to_T(stats_pool, tmp_pool, tr_psum, dest_T, i):
            """LN of xb[:, i, :] (row-major) -> dest_T[:, :, i*128:(i+1)*128]."""
            stats = stats_pool.tile([P, 6], f32, tag="bn_stats")
            nc.vector.bn_stats(stats[:], xb[:, i, :])
            mv = stats_pool.tile([P, 2], f32, tag="bn_aggr")
            nc.vector.bn_aggr(mv[:], stats[:])
            std = stats_pool.tile([P, 1], f32, tag="std")
            nc.scalar.activation(std[:], mv[:, 1:2], AF.Sqrt, bias=eps_t[:],
                                 scale=1.0)
            rstd = stats_pool.tile([P, 1], f32, tag="rstd")
            nc.vector.reciprocal(rstd[:], std[:])
            h_t = tmp_pool.tile([P, D], f32r, tag="h_rm")
            nc.vector.tensor_scalar(out=h_t[:], in0=xb[:, i, :],
                                    scalar1=mv[:, 0:1], scalar2=rstd[:],
                                    op0=OP.subtract, op1=OP.mult)
            for dj in range(DC):
                ps = tr_psum.tile([P, P], f32r, tag="tr")
                nc.tensor.transpose(ps[:], h_t[:, dj * P:(dj + 1) * P], ident[:])
                nc.vector.tensor_copy(dest_T[:, dj, i * P:(i + 1) * P], ps[:])

        # ================= phase A: LN1 + QKV =============================
        with ExitStack() as ph:
            ph_qk = ph.enter_context(tc.tile_pool(name="qk", bufs=1))
            qT = ph_qk.tile([P, DC, S], f32r, tag="qT")
            kT = ph_qk.tile([P, DC, S], f32r, tag="kT")
            v_aug = ph_qk.tile([P, SC, D + 2], f32r, tag="vaug")

            with ExitStack() as pha:
                wA_pool = pha.enter_context(tc.tile_pool(name="wA", bufs=1))
                tmpA = pha.enter_context(tc.tile_pool(name="tmpA", bufs=3))
                statsA = pha.enter_context(tc.tile_pool(name="statsA", bufs=4))
                tr_psA = pha.enter_context(tc.tile_pool(name="trpsA", bufs=2,
                                                        space="PSUM"))
                mm_psA = pha.enter_context(tc.tile_pool(name="mmpsA", bufs=5,
                                                        space="PSUM"))

                wq_t = wA_pool.tile([P, DC, D], f32r)
                nc.sync.dma_start(wq_t[:], wq_r)
                wk_t = wA_pool.tile([P, DC, D], f32r)
                nc.sync.dma_start(wk_t[:], wk_r)
                wvo_t = wA_pool.tile([P, DC, D], f32r)
                nc.sync.dma_start(wvo_t[:], wvo_r)

                for i in range(SC):
                    nc.sync.dma_start(xb[:, i, :], x_r[:, i, :])
                    layer_norm_to_T(statsA, tmpA, tr_psA, hT, i)

                # qT / kT: [dout, s] = Wq'.T h.T
                for (w_t, b_t, dst) in ((wq_t, bq_t, qT), (wk_t, bk_t, kT)):
                    for m in range(DC):
                        for n in range(DC):  # 4 s-tiles of 512
                            ps = mm_psA.tile([P, 512], f32, tag="proj")
                            for k in range(DC):
                                nc.tensor.matmul(
                                    ps[:], w_t[:, k, m * P:(m + 1) * P],
                                    hT[:, k, n * 512:(n + 1) * 512],
                                    start=(k == 0), stop=(k == DC - 1))
                            nc.vector.tensor_scalar_add(
                                dst[:, m, n * 512:(n + 1) * 512], ps[:],
                                b_t[:, m:m + 1])
                # v' row-major: [t, dout] = h @ Wvo'
                for m in range(SC):
                    ps = mm_psA.tile([P, 512], f32, tag="proj")
                    for k in range(DC):
                        nc.tensor.matmul(ps[:], hT[:, k, m * P:(m + 1) * P],
                                         wvo_t[:, k, :],
                                         start=(k == 0), stop=(k == DC - 1))
                    nc.vector.tensor_tensor(v_aug[:, m, 0:D], ps[:], bvo_t[:],
                                            op=OP.add)
                # ones/zero columns for the softmax denominator
                nc.vector.tensor_copy(
                    v_aug[:, :, D:D + 2],
                    ones2[:, None, :].to_broadcast((P, SC, 2)))

            # ============= phase B: attention =============================
            with ExitStack() as phb:
                pT_pool = phb.enter_context(tc.tile_pool(name="pT", bufs=2))
                sc_ps = phb.enter_context(tc.tile_pool(name="scps", bufs=4,
                                                       space="PSUM"))
                a_ps = phb.enter_context(tc.tile_pool(name="aps", bufs=2,
                                                      space="PSUM"))
                rec_pool = phb.enter_context(tc.tile_pool(name="rec", bufs=4))

                for j in range(NB):
                    pT = pT_pool.tile([P, SC, SB], f32r, tag="pT")
                    for m in range(SC):
                        ps = sc_ps.tile([P, SB], f32, tag="sc")
                        for k in range(DC):
                            nc.tensor.matmul(ps[:],
                                             kT[:, k, m * P:(m + 1) * P],
                                             qT[:, k, j * SB:(j + 1) * SB],
                                             start=(k == 0), stop=(k == DC - 1))
                        nc.scalar.activation(pT[:, m, :], ps[:], AF.Exp,
                                             scale=ATTN_SCALE)
                    for c in range(CPB):
                        scn = j * CPB + c
                        pa1 = a_ps.tile([P, 256], f32, tag="pa1")
                        pa2 = a_ps.tile([P, 258], f32, tag="pa2")
                        for m in range(SC):
                            nc.tensor.matmul(pa1[:],
                                             pT[:, m, c * P:(c + 1) * P],
                                             v_aug[:, m, 0:256],
                                             start=(m == 0), stop=(m == SC - 1))
                            nc.tensor.matmul(pa2[:],
                                             pT[:, m, c * P:(c + 1) * P],
                                             v_aug[:, m, 256:514],
                                             start=(m == 0), stop=(m == SC - 1))
                        rec = rec_pool.tile([P, 1], f32, tag="rec")
                        nc.vector.reciprocal(rec[:], pa2[:, 256:257])
                        nc.vector.scalar_tensor_tensor(
                            out=xb[:, scn, 0:256], in0=pa1[:], scalar=rec[:],
                            in1=xb[:, scn, 0:256], op0=OP.mult, op1=OP.add)
                        nc.vector.scalar_tensor_tensor(
                            out=xb[:, scn, 256:512], in0=pa2[:, 0:256],
                            scalar=rec[:], in1=xb[:, scn, 256:512],
                            op0=OP.mult, op1=OP.add)

        # ================= phase C: LN2 + MLP =============================
        with ExitStack() as phc:
            wC_pool = phc.enter_context(tc.tile_pool(name="wC", bufs=1))
            gT_pool = phc.enter_context(tc.tile_pool(name="gT", bufs=2))
            tmpC = phc.enter_context(tc.tile_pool(name="tmpC", bufs=3))
            statsC = phc.enter_context(tc.tile_pool(name="statsC", bufs=4))
            outC = phc.enter_context(tc.tile_pool(name="outC", bufs=3))
            tr_psC = phc.enter_context(tc.tile_pool(name="trpsC", bufs=2,
                                                    space="PSUM"))
            f1_ps = phc.enter_context(tc.tile_pool(name="f1ps", bufs=4,
                                                   space="PSUM"))
            y_ps = phc.enter_context(tc.tile_pool(name="yps", bufs=2,
                                                  space="PSUM"))

            w1_t = wC_pool.tile([P, DC, F], f32r)
            nc.sync.dma_start(w1_t[:], w1_r)
            w2_t = wC_pool.tile([P, FC, D], f32r)
            nc.sync.dma_start(w2_t[:], w2_r)

            h2T = long_pool.tile([P, DC, S], f32r, tag="actT")  # reuses hT slot
            for i in range(SC):
                layer_norm_to_T(statsC, tmpC, tr_psC, h2T, i)

            for jj in range(NB):
                gT = gT_pool.tile([P, FC, SB], f32r, tag="gT")
                for m in range(FC):
                    ps = f1_ps.tile([P, SB], f32, tag="f1")
                    for k in range(DC):
                        nc.tensor.matmul(ps[:], w1_t[:, k, m * P:(m + 1) * P],
                                         h2T[:, k, jj * SB:(jj + 1) * SB],
                                         start=(k == 0), stop=(k == DC - 1))
                    nc.scalar.activation(gT[:, m, :], ps[:], AF.Gelu,
                                         bias=bf_t[:, m:m + 1], scale=1.0)
                for c in range(CPB):
                    scn = jj * CPB + c
                    ps = y_ps.tile([P, D], f32, tag="y")
                    for m in range(FC):
                        nc.tensor.matmul(ps[:], gT[:, m, c * P:(c + 1) * P],
                                         w2_t[:, m, :],
                                         start=(m == 0), stop=(m == FC - 1))
                    y_t = outC.tile([P, D], f32, tag="yout")
                    nc.vector.tensor_tensor(y_t[:], ps[:], xb[:, scn, :],
                                            op=OP.add)
                    nc.vector.tensor_tensor(y_t[:], y_t[:], b2_t[:], op=OP.add)
                    nc.sync.dma_start(out_r[:, scn, :], y_t[:])

    nc.compile()
    return nc


def _fold_weights(inputs):
    """Host-side constant folding (float64): LN affine into weights, Wv@Wo."""
    f64 = {k: np.asarray(v, dtype=np.float64) for k, v in inputs.items()}
    g1, be1, g2, be2 = f64["g1"], f64["be1"], f64["g2"], f64["be2"]
    Wq, Wk, Wv, Wo = f64["Wq"], f64["Wk"], f64["Wv"], f64["Wo"]
    W1, W2 = f64["W1"], f64["W2"]
    b1, b2 = f64["b1"], f64["b2"]
    Wvo = Wv @ Wo
    return {
        "wq": (g1[:, None] * Wq).astype(np.float32),
        "wk": (g1[:, None] * Wk).astype(np.float32),
        "wvo": (g1[:, None] * Wvo).astype(np.float32),
        "w1": (g2[:, None] * W1).astype(np.float32),
        "w2": W2.astype(np.float32),
        "bq": (be1 @ Wq).astype(np.float32),
        "bk": (be1 @ Wk).astype(np.float32),
        "bvo": (be1 @ Wvo).astype(np.float32),
        "bf": (be2 @ W1 + b1).astype(np.float32),
        "b2": b2.astype(np.float32),
    }


def _get_runner():
    """Build (once) a cached jitted SPMD runner over the 8 cores."""
    if "runner" in _CACHE:
        return _CACHE["runner"]

    import jax
    import numpy as _np
    from jax.sharding import Mesh, PartitionSpec, NamedSharding
    from jax.experimental.shard_map import shard_map
    import concourse.mybir as mybir
    from concourse.bass2jax import (_bass_exec_p, install_neuronx_cc_hook,
                                    partition_id_tensor)

    nc = _build()
    install_neuronx_cc_hook()

    partition_name = (nc.partition_id_tensor.name
                      if nc.partition_id_tensor else None)
    in_names, out_names, out_avals, zero_outs = [], [], [], []
    for alloc in nc.m.functions[0].allocations:
        if not isinstance(alloc, mybir.MemoryLocationSet):
            continue
        name = alloc.memorylocations[0].name
        if alloc.kind == "ExternalInput":
            if name != partition_name:
                in_names.append(name)
        elif alloc.kind == "ExternalOutput":
            out_names.append(name)
            shape = tuple(alloc.tensor_shape)
            dtype = mybir.dt.np(alloc.dtype)
            out_avals.append(jax.core.ShapedArray(shape, dtype))
            zero_outs.append(_np.zeros(shape, dtype))
    n_params = len(in_names)
    all_in_names = in_names + out_names
    if partition_name is not None:
        all_in_names = all_in_names + [partition_name]

    def _body(*args):
        operands = list(args)
        if partition_name is not None:
            operands.append(partition_id_tensor())
        outs = _bass_exec_p.bind(
            *operands,
            out_avals=tuple(out_avals),
            in_names=tuple(all_in_names),
            out_names=tuple(out_names),
            lowering_input_output_aliases=(),
            sim_require_finite=True,
            sim_require_nnan=True,
            nc=nc,
        )
        return tuple(outs)

    devices = jax.devices()[:NCORES]
    mesh = Mesh(_np.asarray(devices), ("core",))
    n_all = n_params + len(out_names)
    sharded = jax.jit(
        shard_map(_body, mesh=mesh,
                  in_specs=(PartitionSpec("core"),) * n_all,
                  out_specs=(PartitionSpec("core"),) * len(out_names),
                  check_rep=False),
        keep_unused=True,
    )
    sharding = NamedSharding(mesh, PartitionSpec("core"))
    runner = {
        "sharded": sharded, "sharding": sharding, "in_names": in_names,
        "out_names": out_names, "zero_outs": zero_outs, "jax": jax,
        "np": _np,
    }
    _CACHE["runner"] = runner
    return runner


def _stage(inputs):
    """Shard + fold inputs, return staged device arrays for the runner."""
    r = _get_runner()
    jax, _np = r["jax"], r["np"]
    x = _np.asarray(inputs["x"], dtype=_np.float32)          # [8, 2048, 512]
    folded = _fold_weights(inputs)
    per_core = {"x": [x[c] for c in range(NCORES)]}
    for k, v in folded.items():
        per_core[k] = [v] * NCORES
    concat = []
    for name in r["in_names"]:
        concat.append(_np.concatenate([per_core[name][c] for c in range(NCORES)],
                                      axis=0))
    for z in r["zero_outs"]:
        concat.append(_np.zeros((NCORES * z.shape[0],) + z.shape[1:], z.dtype))
    return [jax.device_put(a, r["sharding"]) for a in concat]


def _run_staged(staged):
    r = _get_runner()
    outs = r["sharded"](*staged)
    return outs


def kernel(**inputs):
    r = _get_runner()
    staged = _stage(inputs)
    outs = _run_staged(staged)
    out = np.asarray(outs[0])                                # [8*2048, 512]
    return out.reshape(NCORES, S, D).astype(np.float32)
